# revision 22
# baseline (speedup 1.0000x reference)
"""VQ codebook kernel for 8 TRN2 NeuronCores.

Computation (matches the reference):
    projected = features @ W.T + b            [B, D]
    projected = l2_normalize(projected)        [B, D]
    sims      = projected @ prototypes.T       [B, K]
    act       = softmax(sims / T, axis=-1)     [B, K]
    best_idx  = argmax(sims, axis=-1)          [B]

Distribution: prototypes are sharded on the K axis across the 8 cores
(tensor-parallel codebook).  Every core computes the full projection
(cheap relative to the similarity matmul), its [B, K/8] similarity
block, exp() with the row 1/(T*norm) scale folded in, a local softmax
numerator + row-sum, then the row-sums are AllReduce'd across cores and
each core normalizes + writes its activation block.  Local (max, argmax)
per row are written out and combined on the host (a 16 KB gather).

Precision/speed: fp32 matmuls on the PE cost 4 cycles/row (two
half-speed passes + per-pass weight reloads).  Instead every matmul
runs as a 3-pass bf16 decomposition — x = hi + lo (hi = bf16(x),
lo = bf16(x - hi)), x.y ~= hi.hi + hi.lo + lo.hi accumulated in fp32
PSUM — which runs at full bf16 PE rate, ~2.6x faster than fp32, with
~2^-18 per-product error (measured end-to-end rel err ~7e-6, same as
fp32).  The hi/lo splits of the kernel inputs are free: the host
passes them as bf16 pairs (same total bytes as fp32).

Row norms are never applied to the projection: the similarity matmul
uses the raw projection and the scalar engine computes
exp(raw * 1/(T*norm_row)) with a per-partition scale, which leaves
argmax and softmax unchanged.

Layouts: the PE contracts along the partition axis, so the host passes
features.T [D, B], W.T [D, E] and the prototype shard transposed
[E, K/8] (hi/lo bf16 pairs).  Host-side transposes/splits are pure
data movement (no FLOPs).
"""

import numpy as np
import ml_dtypes

import concourse.bass as bass
import concourse.bacc as bacc
import concourse.tile as tile
import concourse.mybir as mybir
from concourse import bass_utils

B = 1024          # batch
D = 1024          # feature dim (projection contraction)
E = 1024          # projection output dim (similarity contraction)
K = 32768         # prototypes
N_CORES = 8
KS = K // N_CORES  # 4096 prototypes per core
TEMP = 0.1

P = 128           # partitions
NBLK = 512        # matmul moving width / PSUM bank (fp32 out)
NB = B // P       # 8 batch tiles
NE = E // P       # 8 e tiles
ND = D // P       # 8 d tiles
NJ = KS // NBLK   # 8 k blocks per core
JC = 1            # k blocks per prototype DMA chunk
# batch-tile groups: (start, size)
GROUPS = [(0, 4), (4, 4)]

F32 = mybir.dt.float32
BF16 = mybir.dt.bfloat16
U32 = mybir.dt.uint32

_CACHE = {}


def _build():
    if "nc" in _CACHE:
        return _CACHE["nc"]

    nc = bacc.Bacc("TRN2", target_bir_lowering=False, debug=False,
                   num_devices=N_CORES)

    featT_hi = nc.dram_tensor("featT_hi", [D, B], BF16, kind="ExternalInput").ap()
    featT_lo = nc.dram_tensor("featT_lo", [D, B], BF16, kind="ExternalInput").ap()
    # per-core slice of W.T / bias: core c receives columns [c*128, (c+1)*128)
    WTs_hi = nc.dram_tensor("WTs_hi", [D, P], BF16, kind="ExternalInput").ap()
    WTs_lo = nc.dram_tensor("WTs_lo", [D, P], BF16, kind="ExternalInput").ap()
    bvec = nc.dram_tensor("bvec", [P], F32, kind="ExternalInput").ap()
    protT_hi = nc.dram_tensor("protT_hi", [E, KS], BF16, kind="ExternalInput").ap()
    protT_lo = nc.dram_tensor("protT_lo", [E, KS], BF16, kind="ExternalInput").ap()

    act = nc.dram_tensor("act", [B, KS], F32, kind="ExternalOutput").ap()
    lmax = nc.dram_tensor("lmax", [B, 8], F32, kind="ExternalOutput").ap()
    lidx = nc.dram_tensor("lidx", [B, 8], U32, kind="ExternalOutput").ap()

    AF = mybir.ActivationFunctionType
    ALU = mybir.AluOpType
    AX = mybir.AxisListType

    with tile.TileContext(nc) as tc:
        with tc.tile_pool(name="persist", bufs=1) as pp, \
             tc.tile_pool(name="psum", bufs=4, space="PSUM") as psum, \
             tc.tile_pool(name="psum_n", bufs=2, space="PSUM") as psum_n, \
             tc.tile_pool(name="dram", bufs=2, space="DRAM") as dram:

            btile = pp.tile([P, 1], F32, tag="btile", name="btile")
            nc.sync.dma_start(btile[:], bvec.rearrange("(p o) -> p o", o=1))
            ones = pp.tile([P, 1], F32, tag="ones", name="ones")
            nc.vector.memset(ones[:], 1.0)
            s_tile = pp.tile([P, NB], F32, tag="s_tile", name="s_tile")
            # bf16 hi/lo of the (unnormalized) projection, [e, b] layout
            pj_hi = [pp.tile([P, B], BF16, tag=f"pjh{e}", name=f"pjh{e}")
                     for e in range(NE)]
            pj_lo = [pp.tile([P, B], BF16, tag=f"pjl{e}", name=f"pjl{e}")
                     for e in range(NE)]

            # ---- phase 1+2: projection, sharded over the e axis, with
            # row norms riding along.
            # Core c computes only its own 128-column slice of
            # projT[e, b] = (features @ W.T + b).T (the W.T / bias slices
            # arrive via the per-core input shards), splits it to bf16
            # hi/lo, and an AllGather reassembles the full [E, B]
            # projection on every core — in the partition-major layout the
            # AllGather naturally produces.  The work is pipelined in two
            # b-halves so the second half's matmuls hide the first half's
            # AllGather; batch group 0 (b tiles 0-3) depends only on half 0.
            # Each core's partial row-norms (sum of squares over its 128
            # e's, reduced to [b, 1] by a tiny fp32 matmul against ones)
            # travel inside the same AllGather buffer (f32 pairs bitcast
            # into the bf16 payload), so the full norm is just an 8-way
            # add after the gather.
            HB = B // 2
            with tc.tile_pool(name="inw", bufs=1) as p_in:
                parts_all = p_in.tile([P, NB], F32, tag="parts_all",
                                      name="parts_all")
                fh, fl, wh, wl = [], [], [], []
                for d in range(ND):
                    t = p_in.tile([P, B], BF16, tag=f"fh{d}", name=f"fh{d}")
                    nc.sync.dma_start(t[:], featT_hi[d * P:(d + 1) * P, :])
                    fh.append(t)
                    t = p_in.tile([P, B], BF16, tag=f"fl{d}", name=f"fl{d}")
                    nc.sync.dma_start(t[:], featT_lo[d * P:(d + 1) * P, :])
                    fl.append(t)
                    t = p_in.tile([P, P], BF16, tag=f"wh{d}", name=f"wh{d}")
                    nc.sync.dma_start(t[:], WTs_hi[d * P:(d + 1) * P, :])
                    wh.append(t)
                    t = p_in.tile([P, P], BF16, tag=f"wl{d}", name=f"wl{d}")
                    nc.sync.dma_start(t[:], WTs_lo[d * P:(d + 1) * P, :])
                    wl.append(t)

                for hf in range(2):
                    pj = p_in.tile([P, HB], F32, tag=f"projT{hf}",
                                   name=f"projT{hf}")
                    for h in range(HB // NBLK):
                        hs = slice(hf * HB + h * NBLK,
                                   hf * HB + (h + 1) * NBLK)
                        ps = psum.tile([P, NBLK], F32, tag="ps", name="ps")
                        for d in range(ND):
                            nc.tensor.matmul(ps[:], lhsT=wh[d][:],
                                             rhs=fh[d][:, hs],
                                             start=(d == 0), stop=False)
                            nc.tensor.matmul(ps[:], lhsT=wh[d][:],
                                             rhs=fl[d][:, hs],
                                             start=False, stop=False)
                            nc.tensor.matmul(ps[:], lhsT=wl[d][:],
                                             rhs=fh[d][:, hs],
                                             start=False, stop=(d == ND - 1))
                        nc.vector.tensor_scalar_add(
                            pj[:, h * NBLK:(h + 1) * NBLK], ps[:],
                            btile[:, 0:1])
                    mine_hi = p_in.tile([P, HB], BF16, tag=f"mh{hf}",
                                        name=f"mh{hf}")
                    mine_lo = p_in.tile([P, HB], BF16, tag=f"ml{hf}",
                                        name=f"ml{hf}")
                    # hi-round on the scalar engine, residual on gpsimd —
                    # the DVE's f32->bf16 conversion path was measured to
                    # cost ~20x in end-to-end error vs this pair.
                    nc.scalar.copy(mine_hi[:], pj[:])
                    nc.gpsimd.tensor_sub(mine_lo[:], pj[:], mine_hi[:])
                    # partial norms^2 for this core's 128 e's, [b, 1] layout
                    sqh = p_in.tile([P, HB], F32, tag=f"sqh{hf}",
                                    name=f"sqh{hf}")
                    nc.vector.tensor_mul(sqh[:], pj[:], pj[:])
                    for i in range(4):
                        pnb = psum_n.tile([P, 1], F32, tag="pnb", name="pnb")
                        nc.tensor.matmul(pnb[:],
                                         lhsT=sqh[:, i * P:(i + 1) * P],
                                         rhs=ones[:], start=True, stop=True)
                        nc.vector.tensor_copy(
                            parts_all[:, hf * 4 + i:hf * 4 + i + 1], pnb[:])

                    agin = dram.tile([P, 2 * HB], BF16, tag=f"agin{hf}",
                                     name=f"agin{hf}", bufs=1)
                    agout = dram.tile([NE * P, 2 * HB], BF16,
                                      tag=f"agout{hf}", name=f"agout{hf}",
                                      bufs=1, addr_space="Shared")
                    nc.sync.dma_start(agin[:, 0:HB], mine_hi[:])
                    nc.sync.dma_start(agin[:, HB:2 * HB], mine_lo[:])
                    nc.gpsimd.collective_compute(
                        "AllGather", ALU.bypass,
                        replica_groups=[list(range(N_CORES))],
                        ins=[agin.opt()], outs=[agout.opt()])
                    hsl = slice(hf * HB, (hf + 1) * HB)
                    for e in range(NE):
                        nc.sync.dma_start(pj_hi[e][:, hsl],
                                          agout[e * P:(e + 1) * P, 0:HB])
                        nc.sync.dma_start(pj_lo[e][:, hsl],
                                          agout[e * P:(e + 1) * P, HB:2 * HB])

                # one small f32 AllReduce sums the per-core norm partials
                # (the values must not ride the bf16 AllGather payload —
                # f32 bytes bitcast as bf16 get mangled by the collective
                # datapath on some hops)
                arn_in = dram.tile([P, NB], F32, tag="arn_in", name="arn_in",
                                   bufs=1)
                arn_out = dram.tile([P, NB], F32, tag="arn_out",
                                    name="arn_out", bufs=1)
                nc.sync.dma_start(arn_in[:], parts_all[:])
                nc.gpsimd.collective_compute(
                    "AllReduce", ALU.add,
                    replica_groups=[list(range(N_CORES))],
                    ins=[arn_in.opt()], outs=[arn_out.opt()])
                nsum = p_in.tile([P, NB], F32, tag="nsum", name="nsum")
                nc.sync.dma_start(nsum[:], arn_out[:])
                rec8 = p_in.tile([P, NB], F32, tag="rec8", name="rec8")
                nc.vector.reciprocal(rec8[:], nsum[:])
                # sqrt((1/norm^2) * (1/T^2)) = 1/(T*norm)
                nc.scalar.activation(s_tile[:], rec8[:], AF.Sqrt,
                                     scale=1.0 / (TEMP * TEMP))

            # ---- phase 3: similarities, exp, denominators, argmax
            with tc.tile_pool(name="sims", bufs=7) as p_sims, \
                 tc.tile_pool(name="pt", bufs=2) as p_pt, \
                 tc.tile_pool(name="small", bufs=2) as p_sm:
                for g, (g0, GB) in enumerate(GROUPS):
                    exp_t = [p_sims.tile([P, KS], F32, tag="exp",
                                         name=f"exp_g{g}_{bl}")
                             for bl in range(GB)]
                    dsum_t = [p_sm.tile([P, NJ], F32, tag=f"dsum{bl}",
                                        name=f"dsum{bl}")
                              for bl in range(GB)]
                    # per-j-block top-8 maxima, folded at group end — keeps
                    # the big MAX8 scans off the post-matmul critical path
                    mblk_t = [p_sm.tile([P, 8 * NJ], F32, tag=f"mblk{bl}",
                                        name=f"mblk{bl}")
                              for bl in range(GB)]
                    for jc in range(NJ // JC):
                        pth, ptl = [], []
                        cs = slice(jc * JC * NBLK, (jc + 1) * JC * NBLK)
                        for e in range(NE):
                            t = p_pt.tile([P, JC * NBLK], BF16, tag=f"pth{e}",
                                          name=f"pth{e}")
                            nc.sync.dma_start(t[:], protT_hi[e * P:(e + 1) * P, cs])
                            pth.append(t)
                            t = p_pt.tile([P, JC * NBLK], BF16, tag=f"ptl{e}",
                                          name=f"ptl{e}")
                            nc.sync.dma_start(t[:], protT_lo[e * P:(e + 1) * P, cs])
                            ptl.append(t)
                        for ji in range(JC):
                            j = jc * JC + ji
                            js = slice(ji * NBLK, (ji + 1) * NBLK)
                            for bl in range(GB):
                                bt = g0 + bl
                                bs = slice(bt * P, (bt + 1) * P)
                                ps = psum.tile([P, NBLK], F32, tag="ps",
                                               name="ps")
                                for e in range(NE):
                                    nc.tensor.matmul(ps[:],
                                                     lhsT=pj_hi[e][:, bs],
                                                     rhs=pth[e][:, js],
                                                     start=(e == 0), stop=False)
                                    nc.tensor.matmul(ps[:],
                                                     lhsT=pj_hi[e][:, bs],
                                                     rhs=ptl[e][:, js],
                                                     start=False, stop=False)
                                    nc.tensor.matmul(ps[:],
                                                     lhsT=pj_lo[e][:, bs],
                                                     rhs=pth[e][:, js],
                                                     start=False,
                                                     stop=(e == NE - 1))
                                nc.scalar.activation(
                                    exp_t[bl][:, j * NBLK:(j + 1) * NBLK],
                                    ps[:], AF.Exp,
                                    scale=s_tile[:, bt:bt + 1],
                                    accum_out=dsum_t[bl][:, j:j + 1])
                                nc.vector.max(
                                    mblk_t[bl][:, j * 8:(j + 1) * 8],
                                    exp_t[bl][:, j * NBLK:(j + 1) * NBLK])

                    # local argmax on the unnormalized numerators (argmax is
                    # invariant to the positive per-row normalization, and
                    # the values are cross-core comparable since every core
                    # uses identical row scales) — runs before the collective.
                    for bl in range(GB):
                        bt = g0 + bl
                        m8 = p_sm.tile([P, 8], F32, tag="m8", name="m8")
                        i8 = p_sm.tile([P, 8], U32, tag="i8", name="i8")
                        nc.vector.max(m8[:], mblk_t[bl][:])
                        nc.vector.max_index(i8[:], m8[:], exp_t[bl][:])
                        nc.sync.dma_start(lmax[bt * P:(bt + 1) * P, :], m8[:])
                        nc.sync.dma_start(lidx[bt * P:(bt + 1) * P, :], i8[:])

                    den = p_sm.tile([P, 4], F32, tag="den", name="den")
                    for bl in range(GB):
                        nc.vector.tensor_reduce(den[:, bl:bl + 1], dsum_t[bl][:],
                                                axis=AX.X, op=ALU.add)
                    cin = dram.tile([P, 4], F32, tag="cin", name="cin")
                    cout = dram.tile([P, 4], F32, tag="cout", name="cout")
                    nc.sync.dma_start(cin[:], den[:])
                    nc.gpsimd.collective_compute(
                        "AllReduce", ALU.add,
                        replica_groups=[list(range(N_CORES))],
                        ins=[cin.opt()], outs=[cout.opt()])
                    gd = p_sm.tile([P, 4], F32, tag="gd", name="gd")
                    nc.sync.dma_start(gd[:], cout[:])
                    rd = p_sm.tile([P, 4], F32, tag="rd", name="rd")
                    nc.vector.reciprocal(rd[:], gd[:])

                    for bl in range(GB):
                        bt = g0 + bl
                        # normalization scale on the scalar engine — the
                        # vector engine is running the FIND_INDEX8 scans
                        nc.scalar.mul(exp_t[bl][:], exp_t[bl][:],
                                      rd[:, bl:bl + 1])
                        nc.sync.dma_start(act[bt * P:(bt + 1) * P, :],
                                          exp_t[bl][:])

    nc.compile()
    _CACHE["nc"] = nc
    return nc


def _split_bf16(x):
    hi = x.astype(ml_dtypes.bfloat16)
    lo = (x - hi.astype(np.float32)).astype(ml_dtypes.bfloat16)
    return np.ascontiguousarray(hi), np.ascontiguousarray(lo)


def kernel(features, W, b, prototypes, _run_kwargs=None):
    nc = _build()

    featT = np.asarray(features, dtype=np.float32).T
    WT = np.asarray(W, dtype=np.float32).T
    bvec = np.asarray(b, dtype=np.float32)
    prototypes = np.asarray(prototypes, dtype=np.float32)

    fh, fl = _split_bf16(featT)

    in_maps = []
    for c in range(N_CORES):
        ph, pl = _split_bf16(prototypes[c * KS:(c + 1) * KS].T)
        wsh, wsl = _split_bf16(WT[:, c * P:(c + 1) * P])
        in_maps.append({"featT_hi": fh, "featT_lo": fl,
                        "WTs_hi": wsh, "WTs_lo": wsl,
                        "bvec": np.ascontiguousarray(bvec[c * P:(c + 1) * P]),
                        "protT_hi": ph, "protT_lo": pl})

    res = bass_utils.run_bass_kernel_spmd(
        nc, in_maps, core_ids=list(range(N_CORES)), **(_run_kwargs or {}))
    if _run_kwargs:
        _CACHE["last_result"] = res

    act = np.concatenate([res.results[c]["act"] for c in range(N_CORES)],
                         axis=1)
    lmax = np.stack([res.results[c]["lmax"][:, 0] for c in range(N_CORES)])
    lidx = np.stack([res.results[c]["lidx"][:, 0] for c in range(N_CORES)])
    best_core = np.argmax(lmax, axis=0)                       # [B]
    rows = np.arange(B)
    best_idx = (best_core * KS + lidx[best_core, rows]).astype(np.int32)
    return act, best_idx


# revision 23
# speedup vs baseline: 1.0004x; 1.0004x over previous
"""VQ codebook kernel for 8 TRN2 NeuronCores.

Computation (matches the reference):
    projected = features @ W.T + b            [B, D]
    projected = l2_normalize(projected)        [B, D]
    sims      = projected @ prototypes.T       [B, K]
    act       = softmax(sims / T, axis=-1)     [B, K]
    best_idx  = argmax(sims, axis=-1)          [B]

Distribution: prototypes are sharded on the K axis across the 8 cores
(tensor-parallel codebook).  Every core computes the full projection
(cheap relative to the similarity matmul), its [B, K/8] similarity
block, exp() with the row 1/(T*norm) scale folded in, a local softmax
numerator + row-sum, then the row-sums are AllReduce'd across cores and
each core normalizes + writes its activation block.  Local (max, argmax)
per row are written out and combined on the host (a 16 KB gather).

Precision/speed: fp32 matmuls on the PE cost 4 cycles/row (two
half-speed passes + per-pass weight reloads).  Instead every matmul
runs as a 3-pass bf16 decomposition — x = hi + lo (hi = bf16(x),
lo = bf16(x - hi)), x.y ~= hi.hi + hi.lo + lo.hi accumulated in fp32
PSUM — which runs at full bf16 PE rate, ~2.6x faster than fp32, with
~2^-18 per-product error (measured end-to-end rel err ~7e-6, same as
fp32).  The hi/lo splits of the kernel inputs are free: the host
passes them as bf16 pairs (same total bytes as fp32).

Row norms are never applied to the projection: the similarity matmul
uses the raw projection and the scalar engine computes
exp(raw * 1/(T*norm_row)) with a per-partition scale, which leaves
argmax and softmax unchanged.

Layouts: the PE contracts along the partition axis, so the host passes
features.T [D, B], W.T [D, E] and the prototype shard transposed
[E, K/8] (hi/lo bf16 pairs).  Host-side transposes/splits are pure
data movement (no FLOPs).
"""

import numpy as np
import ml_dtypes

import concourse.bass as bass
import concourse.bacc as bacc
import concourse.tile as tile
import concourse.mybir as mybir
from concourse import bass_utils

B = 1024          # batch
D = 1024          # feature dim (projection contraction)
E = 1024          # projection output dim (similarity contraction)
K = 32768         # prototypes
N_CORES = 8
KS = K // N_CORES  # 4096 prototypes per core
TEMP = 0.1

P = 128           # partitions
NBLK = 512        # matmul moving width / PSUM bank (fp32 out)
NB = B // P       # 8 batch tiles
NE = E // P       # 8 e tiles
ND = D // P       # 8 d tiles
NJ = KS // NBLK   # 8 k blocks per core
JC = 1            # k blocks per prototype DMA chunk
# batch-tile groups: (start, size)
GROUPS = [(0, 4), (4, 4)]

F32 = mybir.dt.float32
BF16 = mybir.dt.bfloat16
U32 = mybir.dt.uint32

_CACHE = {}


def _build():
    if "nc" in _CACHE:
        return _CACHE["nc"]

    nc = bacc.Bacc("TRN2", target_bir_lowering=False, debug=False,
                   num_devices=N_CORES)

    featT_hi = nc.dram_tensor("featT_hi", [D, B], BF16, kind="ExternalInput").ap()
    featT_lo = nc.dram_tensor("featT_lo", [D, B], BF16, kind="ExternalInput").ap()
    # per-core slice of W.T / bias: core c receives columns [c*128, (c+1)*128)
    WTs_hi = nc.dram_tensor("WTs_hi", [D, P], BF16, kind="ExternalInput").ap()
    WTs_lo = nc.dram_tensor("WTs_lo", [D, P], BF16, kind="ExternalInput").ap()
    bvec = nc.dram_tensor("bvec", [P], F32, kind="ExternalInput").ap()
    protT_hi = nc.dram_tensor("protT_hi", [E, KS], BF16, kind="ExternalInput").ap()
    protT_lo = nc.dram_tensor("protT_lo", [E, KS], BF16, kind="ExternalInput").ap()

    act = nc.dram_tensor("act", [B, KS], F32, kind="ExternalOutput").ap()
    lmax = nc.dram_tensor("lmax", [B, 8], F32, kind="ExternalOutput").ap()
    lidx = nc.dram_tensor("lidx", [B, 8], U32, kind="ExternalOutput").ap()

    AF = mybir.ActivationFunctionType
    ALU = mybir.AluOpType
    AX = mybir.AxisListType

    with tile.TileContext(nc) as tc:
        with tc.tile_pool(name="persist", bufs=1) as pp, \
             tc.tile_pool(name="psum", bufs=4, space="PSUM") as psum, \
             tc.tile_pool(name="psum_n", bufs=2, space="PSUM") as psum_n, \
             tc.tile_pool(name="dram", bufs=2, space="DRAM") as dram:

            # dummy warmup collective: the first collective in a NEFF pays
            # ~35us of ncfw wake/init before bytes move; firing a tiny one
            # immediately hides that under the projection.
            wu_in = dram.tile([P, 1], F32, tag="wu_in", name="wu_in", bufs=1)
            wu_out = dram.tile([P, 1], F32, tag="wu_out", name="wu_out",
                               bufs=1)
            nc.gpsimd.collective_compute(
                "AllReduce", mybir.AluOpType.add,
                replica_groups=[list(range(N_CORES))],
                ins=[wu_in.opt()], outs=[wu_out.opt()])

            btile = pp.tile([P, 1], F32, tag="btile", name="btile")
            nc.sync.dma_start(btile[:], bvec.rearrange("(p o) -> p o", o=1))
            ones = pp.tile([P, 1], F32, tag="ones", name="ones")
            nc.vector.memset(ones[:], 1.0)
            s_tile = pp.tile([P, NB], F32, tag="s_tile", name="s_tile")
            # bf16 hi/lo of the (unnormalized) projection, [e, b] layout
            pj_hi = [pp.tile([P, B], BF16, tag=f"pjh{e}", name=f"pjh{e}")
                     for e in range(NE)]
            pj_lo = [pp.tile([P, B], BF16, tag=f"pjl{e}", name=f"pjl{e}")
                     for e in range(NE)]

            # ---- phase 1+2: projection, sharded over the e axis, with
            # row norms riding along.
            # Core c computes only its own 128-column slice of
            # projT[e, b] = (features @ W.T + b).T (the W.T / bias slices
            # arrive via the per-core input shards), splits it to bf16
            # hi/lo, and an AllGather reassembles the full [E, B]
            # projection on every core — in the partition-major layout the
            # AllGather naturally produces.  The work is pipelined in two
            # b-halves so the second half's matmuls hide the first half's
            # AllGather; batch group 0 (b tiles 0-3) depends only on half 0.
            # Each core's partial row-norms (sum of squares over its 128
            # e's, reduced to [b, 1] by a tiny fp32 matmul against ones)
            # travel inside the same AllGather buffer (f32 pairs bitcast
            # into the bf16 payload), so the full norm is just an 8-way
            # add after the gather.
            HB = B // 2
            with tc.tile_pool(name="inw", bufs=1) as p_in:
                parts_all = p_in.tile([P, NB], F32, tag="parts_all",
                                      name="parts_all")
                ag_bufs = []

                def _emit_gather(hf):
                    agin, agout = ag_bufs[hf]
                    nc.gpsimd.collective_compute(
                        "AllGather", ALU.bypass,
                        replica_groups=[list(range(N_CORES))],
                        ins=[agin.opt()], outs=[agout.opt()])
                    hsl = slice(hf * HB, (hf + 1) * HB)
                    for e in range(NE):
                        nc.sync.dma_start(pj_hi[e][:, hsl],
                                          agout[e * P:(e + 1) * P, 0:HB])
                        nc.sync.dma_start(pj_lo[e][:, hsl],
                                          agout[e * P:(e + 1) * P, HB:2 * HB])
                fh, fl, wh, wl = [], [], [], []
                for d in range(ND):
                    t = p_in.tile([P, B], BF16, tag=f"fh{d}", name=f"fh{d}")
                    nc.sync.dma_start(t[:], featT_hi[d * P:(d + 1) * P, :])
                    fh.append(t)
                    t = p_in.tile([P, B], BF16, tag=f"fl{d}", name=f"fl{d}")
                    nc.sync.dma_start(t[:], featT_lo[d * P:(d + 1) * P, :])
                    fl.append(t)
                    t = p_in.tile([P, P], BF16, tag=f"wh{d}", name=f"wh{d}")
                    nc.sync.dma_start(t[:], WTs_hi[d * P:(d + 1) * P, :])
                    wh.append(t)
                    t = p_in.tile([P, P], BF16, tag=f"wl{d}", name=f"wl{d}")
                    nc.sync.dma_start(t[:], WTs_lo[d * P:(d + 1) * P, :])
                    wl.append(t)

                for hf in range(2):
                    pj = p_in.tile([P, HB], F32, tag=f"projT{hf}",
                                   name=f"projT{hf}")
                    for h in range(HB // NBLK):
                        hs = slice(hf * HB + h * NBLK,
                                   hf * HB + (h + 1) * NBLK)
                        ps = psum.tile([P, NBLK], F32, tag="ps", name="ps")
                        for d in range(ND):
                            nc.tensor.matmul(ps[:], lhsT=wh[d][:],
                                             rhs=fh[d][:, hs],
                                             start=(d == 0), stop=False)
                            nc.tensor.matmul(ps[:], lhsT=wh[d][:],
                                             rhs=fl[d][:, hs],
                                             start=False, stop=False)
                            nc.tensor.matmul(ps[:], lhsT=wl[d][:],
                                             rhs=fh[d][:, hs],
                                             start=False, stop=(d == ND - 1))
                        nc.vector.tensor_scalar_add(
                            pj[:, h * NBLK:(h + 1) * NBLK], ps[:],
                            btile[:, 0:1])
                    mine_hi = p_in.tile([P, HB], BF16, tag=f"mh{hf}",
                                        name=f"mh{hf}")
                    mine_lo = p_in.tile([P, HB], BF16, tag=f"ml{hf}",
                                        name=f"ml{hf}")
                    # hi-round on the scalar engine, residual on gpsimd —
                    # the DVE's f32->bf16 conversion path was measured to
                    # cost ~20x in end-to-end error vs this pair.
                    nc.scalar.copy(mine_hi[:], pj[:])
                    nc.gpsimd.tensor_sub(mine_lo[:], pj[:], mine_hi[:])
                    # partial norms^2 for this core's 128 e's, [b, 1] layout
                    sqh = p_in.tile([P, HB], F32, tag=f"sqh{hf}",
                                    name=f"sqh{hf}")
                    nc.vector.tensor_mul(sqh[:], pj[:], pj[:])
                    for i in range(4):
                        pnb = psum_n.tile([P, 1], F32, tag="pnb", name="pnb")
                        nc.tensor.matmul(pnb[:],
                                         lhsT=sqh[:, i * P:(i + 1) * P],
                                         rhs=ones[:], start=True, stop=True)
                        nc.vector.tensor_copy(
                            parts_all[:, hf * 4 + i:hf * 4 + i + 1], pnb[:])

                    agin = dram.tile([P, 2 * HB], BF16, tag=f"agin{hf}",
                                     name=f"agin{hf}", bufs=1)
                    agout = dram.tile([NE * P, 2 * HB], BF16,
                                      tag=f"agout{hf}", name=f"agout{hf}",
                                      bufs=1, addr_space="Shared")
                    nc.sync.dma_start(agin[:, 0:HB], mine_hi[:])
                    nc.sync.dma_start(agin[:, HB:2 * HB], mine_lo[:])
                    ag_bufs.append((agin, agout))
                    if hf == 0:
                        _emit_gather(0)

                # the second half's AllGather is not urgent (batch group 1
                # runs ~200us later), so the norm AllReduce goes first on
                # the collective queue
                # one small f32 AllReduce sums the per-core norm partials
                # (the values must not ride the bf16 AllGather payload —
                # f32 bytes bitcast as bf16 get mangled by the collective
                # datapath on some hops)
                arn_in = dram.tile([P, NB], F32, tag="arn_in", name="arn_in",
                                   bufs=1)
                arn_out = dram.tile([P, NB], F32, tag="arn_out",
                                    name="arn_out", bufs=1)
                nc.sync.dma_start(arn_in[:], parts_all[:])
                nc.gpsimd.collective_compute(
                    "AllReduce", ALU.add,
                    replica_groups=[list(range(N_CORES))],
                    ins=[arn_in.opt()], outs=[arn_out.opt()])
                nsum = p_in.tile([P, NB], F32, tag="nsum", name="nsum")
                nc.sync.dma_start(nsum[:], arn_out[:])
                rec8 = p_in.tile([P, NB], F32, tag="rec8", name="rec8")
                nc.vector.reciprocal(rec8[:], nsum[:])
                # sqrt((1/norm^2) * (1/T^2)) = 1/(T*norm)
                nc.scalar.activation(s_tile[:], rec8[:], AF.Sqrt,
                                     scale=1.0 / (TEMP * TEMP))
                _emit_gather(1)

            # ---- phase 3: similarities, exp, denominators, argmax
            with tc.tile_pool(name="sims", bufs=7) as p_sims, \
                 tc.tile_pool(name="pt", bufs=2) as p_pt, \
                 tc.tile_pool(name="small", bufs=2) as p_sm:
                for g, (g0, GB) in enumerate(GROUPS):
                    exp_t = [p_sims.tile([P, KS], F32, tag="exp",
                                         name=f"exp_g{g}_{bl}")
                             for bl in range(GB)]
                    dsum_t = [p_sm.tile([P, NJ], F32, tag=f"dsum{bl}",
                                        name=f"dsum{bl}")
                              for bl in range(GB)]
                    # per-j-block top-8 maxima, folded at group end — keeps
                    # the big MAX8 scans off the post-matmul critical path
                    mblk_t = [p_sm.tile([P, 8 * NJ], F32, tag=f"mblk{bl}",
                                        name=f"mblk{bl}")
                              for bl in range(GB)]
                    for jc in range(NJ // JC):
                        pth, ptl = [], []
                        cs = slice(jc * JC * NBLK, (jc + 1) * JC * NBLK)
                        for e in range(NE):
                            t = p_pt.tile([P, JC * NBLK], BF16, tag=f"pth{e}",
                                          name=f"pth{e}")
                            nc.sync.dma_start(t[:], protT_hi[e * P:(e + 1) * P, cs])
                            pth.append(t)
                            t = p_pt.tile([P, JC * NBLK], BF16, tag=f"ptl{e}",
                                          name=f"ptl{e}")
                            nc.sync.dma_start(t[:], protT_lo[e * P:(e + 1) * P, cs])
                            ptl.append(t)
                        for ji in range(JC):
                            j = jc * JC + ji
                            js = slice(ji * NBLK, (ji + 1) * NBLK)
                            for bl in range(GB):
                                bt = g0 + bl
                                bs = slice(bt * P, (bt + 1) * P)
                                ps = psum.tile([P, NBLK], F32, tag="ps",
                                               name="ps")
                                for e in range(NE):
                                    nc.tensor.matmul(ps[:],
                                                     lhsT=pj_hi[e][:, bs],
                                                     rhs=pth[e][:, js],
                                                     start=(e == 0), stop=False)
                                    nc.tensor.matmul(ps[:],
                                                     lhsT=pj_hi[e][:, bs],
                                                     rhs=ptl[e][:, js],
                                                     start=False, stop=False)
                                    nc.tensor.matmul(ps[:],
                                                     lhsT=pj_lo[e][:, bs],
                                                     rhs=pth[e][:, js],
                                                     start=False,
                                                     stop=(e == NE - 1))
                                nc.scalar.activation(
                                    exp_t[bl][:, j * NBLK:(j + 1) * NBLK],
                                    ps[:], AF.Exp,
                                    scale=s_tile[:, bt:bt + 1],
                                    accum_out=dsum_t[bl][:, j:j + 1])
                                nc.vector.max(
                                    mblk_t[bl][:, j * 8:(j + 1) * 8],
                                    exp_t[bl][:, j * NBLK:(j + 1) * NBLK])

                    # local argmax on the unnormalized numerators (argmax is
                    # invariant to the positive per-row normalization, and
                    # the values are cross-core comparable since every core
                    # uses identical row scales) — runs before the collective.
                    for bl in range(GB):
                        bt = g0 + bl
                        m8 = p_sm.tile([P, 8], F32, tag="m8", name="m8")
                        i8 = p_sm.tile([P, 8], U32, tag="i8", name="i8")
                        nc.vector.max(m8[:], mblk_t[bl][:])
                        nc.vector.max_index(i8[:], m8[:], exp_t[bl][:])
                        nc.sync.dma_start(lmax[bt * P:(bt + 1) * P, :], m8[:])
                        nc.sync.dma_start(lidx[bt * P:(bt + 1) * P, :], i8[:])

                    den = p_sm.tile([P, 4], F32, tag="den", name="den")
                    for bl in range(GB):
                        nc.vector.tensor_reduce(den[:, bl:bl + 1], dsum_t[bl][:],
                                                axis=AX.X, op=ALU.add)
                    cin = dram.tile([P, 4], F32, tag="cin", name="cin")
                    cout = dram.tile([P, 4], F32, tag="cout", name="cout")
                    nc.sync.dma_start(cin[:], den[:])
                    nc.gpsimd.collective_compute(
                        "AllReduce", ALU.add,
                        replica_groups=[list(range(N_CORES))],
                        ins=[cin.opt()], outs=[cout.opt()])
                    gd = p_sm.tile([P, 4], F32, tag="gd", name="gd")
                    nc.sync.dma_start(gd[:], cout[:])
                    rd = p_sm.tile([P, 4], F32, tag="rd", name="rd")
                    nc.vector.reciprocal(rd[:], gd[:])

                    for bl in range(GB):
                        bt = g0 + bl
                        # normalization scale on the scalar engine — the
                        # vector engine is running the FIND_INDEX8 scans
                        nc.scalar.mul(exp_t[bl][:], exp_t[bl][:],
                                      rd[:, bl:bl + 1])
                        nc.sync.dma_start(act[bt * P:(bt + 1) * P, :],
                                          exp_t[bl][:])

    nc.compile()
    _CACHE["nc"] = nc
    return nc


def _split_bf16(x):
    hi = x.astype(ml_dtypes.bfloat16)
    lo = (x - hi.astype(np.float32)).astype(ml_dtypes.bfloat16)
    return np.ascontiguousarray(hi), np.ascontiguousarray(lo)


def kernel(features, W, b, prototypes, _run_kwargs=None):
    nc = _build()

    featT = np.asarray(features, dtype=np.float32).T
    WT = np.asarray(W, dtype=np.float32).T
    bvec = np.asarray(b, dtype=np.float32)
    prototypes = np.asarray(prototypes, dtype=np.float32)

    fh, fl = _split_bf16(featT)

    in_maps = []
    for c in range(N_CORES):
        ph, pl = _split_bf16(prototypes[c * KS:(c + 1) * KS].T)
        wsh, wsl = _split_bf16(WT[:, c * P:(c + 1) * P])
        in_maps.append({"featT_hi": fh, "featT_lo": fl,
                        "WTs_hi": wsh, "WTs_lo": wsl,
                        "bvec": np.ascontiguousarray(bvec[c * P:(c + 1) * P]),
                        "protT_hi": ph, "protT_lo": pl})

    res = bass_utils.run_bass_kernel_spmd(
        nc, in_maps, core_ids=list(range(N_CORES)), **(_run_kwargs or {}))
    if _run_kwargs:
        _CACHE["last_result"] = res

    act = np.concatenate([res.results[c]["act"] for c in range(N_CORES)],
                         axis=1)
    lmax = np.stack([res.results[c]["lmax"][:, 0] for c in range(N_CORES)])
    lidx = np.stack([res.results[c]["lidx"][:, 0] for c in range(N_CORES)])
    best_core = np.argmax(lmax, axis=0)                       # [B]
    rows = np.arange(B)
    best_idx = (best_core * KS + lidx[best_core, rows]).astype(np.int32)
    return act, best_idx


# revision 25
# speedup vs baseline: 1.0056x; 1.0051x over previous
"""VQ codebook kernel for 8 TRN2 NeuronCores.

Computation (matches the reference):
    projected = features @ W.T + b            [B, D]
    projected = l2_normalize(projected)        [B, D]
    sims      = projected @ prototypes.T       [B, K]
    act       = softmax(sims / T, axis=-1)     [B, K]
    best_idx  = argmax(sims, axis=-1)          [B]

Distribution: prototypes are sharded on the K axis across the 8 cores
(tensor-parallel codebook).  Every core computes the full projection
(cheap relative to the similarity matmul), its [B, K/8] similarity
block, exp() with the row 1/(T*norm) scale folded in, a local softmax
numerator + row-sum, then the row-sums are AllReduce'd across cores and
each core normalizes + writes its activation block.  Local (max, argmax)
per row are written out and combined on the host (a 16 KB gather).

Precision/speed: fp32 matmuls on the PE cost 4 cycles/row (two
half-speed passes + per-pass weight reloads).  Instead every matmul
runs as a 3-pass bf16 decomposition — x = hi + lo (hi = bf16(x),
lo = bf16(x - hi)), x.y ~= hi.hi + hi.lo + lo.hi accumulated in fp32
PSUM — which runs at full bf16 PE rate, ~2.6x faster than fp32, with
~2^-18 per-product error (measured end-to-end rel err ~7e-6, same as
fp32).  The hi/lo splits of the kernel inputs are free: the host
passes them as bf16 pairs (same total bytes as fp32).

Row norms are never applied to the projection: the similarity matmul
uses the raw projection and the scalar engine computes
exp(raw * 1/(T*norm_row)) with a per-partition scale, which leaves
argmax and softmax unchanged.

Layouts: the PE contracts along the partition axis, so the host passes
features.T [D, B], W.T [D, E] and the prototype shard transposed
[E, K/8] (hi/lo bf16 pairs).  Host-side transposes/splits are pure
data movement (no FLOPs).
"""

import numpy as np
import ml_dtypes

import concourse.bass as bass
import concourse.bacc as bacc
import concourse.tile as tile
import concourse.mybir as mybir
from concourse import bass_utils

B = 1024          # batch
D = 1024          # feature dim (projection contraction)
E = 1024          # projection output dim (similarity contraction)
K = 32768         # prototypes
N_CORES = 8
KS = K // N_CORES  # 4096 prototypes per core
TEMP = 0.1

P = 128           # partitions
NBLK = 512        # matmul moving width / PSUM bank (fp32 out)
NB = B // P       # 8 batch tiles
NE = E // P       # 8 e tiles
ND = D // P       # 8 d tiles
NJ = KS // NBLK   # 8 k blocks per core
JC = 1            # k blocks per prototype DMA chunk
# batch-tile groups: (start, size)
GROUPS = [(0, 4), (4, 4)]

F32 = mybir.dt.float32
BF16 = mybir.dt.bfloat16
U32 = mybir.dt.uint32

_CACHE = {}



def _build():
    if "nc" in _CACHE:
        return _CACHE["nc"]

    nc = bacc.Bacc("TRN2", target_bir_lowering=False, debug=False,
                   num_devices=N_CORES)

    featT_hi = nc.dram_tensor("featT_hi", [D, B], BF16, kind="ExternalInput").ap()
    featT_lo = nc.dram_tensor("featT_lo", [D, B], BF16, kind="ExternalInput").ap()
    # per-core slice of W.T / bias: core c receives columns [c*128, (c+1)*128)
    WTs_hi = nc.dram_tensor("WTs_hi", [D, P], BF16, kind="ExternalInput").ap()
    WTs_lo = nc.dram_tensor("WTs_lo", [D, P], BF16, kind="ExternalInput").ap()
    bvec = nc.dram_tensor("bvec", [P], F32, kind="ExternalInput").ap()
    protT_hi = nc.dram_tensor("protT_hi", [E, KS], BF16, kind="ExternalInput").ap()
    protT_lo = nc.dram_tensor("protT_lo", [E, KS], BF16, kind="ExternalInput").ap()

    act = nc.dram_tensor("act", [B, KS], F32, kind="ExternalOutput").ap()
    lmax = nc.dram_tensor("lmax", [B, 8], F32, kind="ExternalOutput").ap()
    lidx = nc.dram_tensor("lidx", [B, 8], U32, kind="ExternalOutput").ap()

    AF = mybir.ActivationFunctionType
    ALU = mybir.AluOpType
    AX = mybir.AxisListType

    with tile.TileContext(nc) as tc:
        with tc.tile_pool(name="persist", bufs=1) as pp, \
             tc.tile_pool(name="psum", bufs=4, space="PSUM") as psum, \
             tc.tile_pool(name="psum_n", bufs=2, space="PSUM") as psum_n, \
             tc.tile_pool(name="dram", bufs=2, space="DRAM") as dram:

            # dummy warmup collective: the first collective in a NEFF pays
            # ~35us of ncfw wake/init before bytes move; firing a tiny one
            # immediately hides that under the projection.
            wu_in = dram.tile([P, 1], F32, tag="wu_in2", name="wu_in2", bufs=1)
            wu_out = dram.tile([P, 1], F32, tag="wu_out", name="wu_out",
                               bufs=1)
            nc.gpsimd.collective_compute(
                "AllReduce", mybir.AluOpType.add,
                replica_groups=[list(range(N_CORES))],
                ins=[wu_in.opt()], outs=[wu_out.opt()])

            btile = pp.tile([P, 1], F32, tag="btile", name="btile")
            nc.sync.dma_start(btile[:], bvec.rearrange("(p o) -> p o", o=1))
            ones = pp.tile([P, 1], F32, tag="ones", name="ones")
            nc.vector.memset(ones[:], 1.0)
            s_tile = pp.tile([P, NB], F32, tag="s_tile", name="s_tile")
            # bf16 hi/lo of the (unnormalized) projection, [e, b] layout
            pj_hi = [pp.tile([P, B], BF16, tag=f"pjh{e}", name=f"pjh{e}")
                     for e in range(NE)]
            pj_lo = [pp.tile([P, B], BF16, tag=f"pjl{e}", name=f"pjl{e}")
                     for e in range(NE)]

            # ---- phase 1+2: projection, sharded over the e axis, with
            # row norms riding along.
            # Core c computes only its own 128-column slice of
            # projT[e, b] = (features @ W.T + b).T (the W.T / bias slices
            # arrive via the per-core input shards), splits it to bf16
            # hi/lo, and an AllGather reassembles the full [E, B]
            # projection on every core — in the partition-major layout the
            # AllGather naturally produces.  The work is pipelined in two
            # b-halves so the second half's matmuls hide the first half's
            # AllGather; batch group 0 (b tiles 0-3) depends only on half 0.
            # Each core's partial row-norms (sum of squares over its 128
            # e's, reduced to [b, 1] by a tiny fp32 matmul against ones)
            # travel inside the same AllGather buffer (f32 pairs bitcast
            # into the bf16 payload), so the full norm is just an 8-way
            # add after the gather.
            HB = B // 2
            with tc.tile_pool(name="inw", bufs=1) as p_in:
                parts_all = p_in.tile([P, NB], F32, tag="parts_all",
                                      name="parts_all")
                ag_bufs = []

                def _emit_gather(hf):
                    agin, agout = ag_bufs[hf]
                    nc.gpsimd.collective_compute(
                        "AllGather", ALU.bypass,
                        replica_groups=[list(range(N_CORES))],
                        ins=[agin.opt()], outs=[agout.opt()])
                    hsl = slice(hf * HB, (hf + 1) * HB)
                    for e in range(NE):
                        nc.sync.dma_start(pj_hi[e][:, hsl],
                                          agout[e * P:(e + 1) * P, 0:HB])
                        nc.sync.dma_start(pj_lo[e][:, hsl],
                                          agout[e * P:(e + 1) * P, HB:2 * HB])
                fh, fl, wh, wl = [], [], [], []
                for d in range(ND):
                    t = p_in.tile([P, B], BF16, tag=f"fh{d}", name=f"fh{d}")
                    nc.sync.dma_start(t[:], featT_hi[d * P:(d + 1) * P, :])
                    fh.append(t)
                    t = p_in.tile([P, B], BF16, tag=f"fl{d}", name=f"fl{d}")
                    nc.sync.dma_start(t[:], featT_lo[d * P:(d + 1) * P, :])
                    fl.append(t)
                    t = p_in.tile([P, P], BF16, tag=f"wh{d}", name=f"wh{d}")
                    nc.sync.dma_start(t[:], WTs_hi[d * P:(d + 1) * P, :])
                    wh.append(t)
                    t = p_in.tile([P, P], BF16, tag=f"wl{d}", name=f"wl{d}")
                    nc.sync.dma_start(t[:], WTs_lo[d * P:(d + 1) * P, :])
                    wl.append(t)

                for hf in range(2):
                    pj = p_in.tile([P, HB], F32, tag=f"projT{hf}",
                                   name=f"projT{hf}")
                    for h in range(HB // NBLK):
                        hs = slice(hf * HB + h * NBLK,
                                   hf * HB + (h + 1) * NBLK)
                        ps = psum.tile([P, NBLK], F32, tag="ps", name="ps")
                        for d in range(ND):
                            nc.tensor.matmul(ps[:], lhsT=wh[d][:],
                                             rhs=fh[d][:, hs],
                                             start=(d == 0), stop=False)
                            nc.tensor.matmul(ps[:], lhsT=wh[d][:],
                                             rhs=fl[d][:, hs],
                                             start=False, stop=False)
                            nc.tensor.matmul(ps[:], lhsT=wl[d][:],
                                             rhs=fh[d][:, hs],
                                             start=False, stop=(d == ND - 1))
                        nc.vector.tensor_scalar_add(
                            pj[:, h * NBLK:(h + 1) * NBLK], ps[:],
                            btile[:, 0:1])
                    mine_hi = p_in.tile([P, HB], BF16, tag=f"mh{hf}",
                                        name=f"mh{hf}")
                    mine_lo = p_in.tile([P, HB], BF16, tag=f"ml{hf}",
                                        name=f"ml{hf}")
                    # hi-round on the scalar engine, residual on gpsimd —
                    # the DVE's f32->bf16 conversion path was measured to
                    # cost ~20x in end-to-end error vs this pair.
                    nc.scalar.copy(mine_hi[:], pj[:])
                    nc.gpsimd.tensor_sub(mine_lo[:], pj[:], mine_hi[:])
                    # partial norms^2 for this core's 128 e's, [b, 1] layout
                    sqh = p_in.tile([P, HB], F32, tag=f"sqh{hf}",
                                    name=f"sqh{hf}")
                    nc.vector.tensor_mul(sqh[:], pj[:], pj[:])
                    for i in range(4):
                        pnb = psum_n.tile([P, 1], F32, tag="pnb", name="pnb")
                        nc.tensor.matmul(pnb[:],
                                         lhsT=sqh[:, i * P:(i + 1) * P],
                                         rhs=ones[:], start=True, stop=True)
                        nc.vector.tensor_copy(
                            parts_all[:, hf * 4 + i:hf * 4 + i + 1], pnb[:])

                    agin = dram.tile([P, 2 * HB], BF16, tag=f"agin{hf}",
                                     name=f"agin{hf}", bufs=1)
                    agout = dram.tile([NE * P, 2 * HB], BF16,
                                      tag=f"agout{hf}", name=f"agout{hf}",
                                      bufs=1, addr_space="Shared")
                    nc.sync.dma_start(agin[:, 0:HB], mine_hi[:])
                    nc.sync.dma_start(agin[:, HB:2 * HB], mine_lo[:])
                    ag_bufs.append((agin, agout))
                    if hf == 0:
                        _emit_gather(0)

                # the second half's AllGather is not urgent (batch group 1
                # runs ~200us later), so the norm AllReduce goes first on
                # the collective queue
                # one small f32 AllReduce sums the per-core norm partials
                # (the values must not ride the bf16 AllGather payload —
                # f32 bytes bitcast as bf16 get mangled by the collective
                # datapath on some hops)
                arn_in = dram.tile([P, NB], F32, tag="arn_in", name="arn_in",
                                   bufs=1)
                arn_out = dram.tile([P, NB], F32, tag="arn_out",
                                    name="arn_out", bufs=1)
                nc.sync.dma_start(arn_in[:], parts_all[:])
                nc.gpsimd.collective_compute(
                    "AllReduce", ALU.add,
                    replica_groups=[list(range(N_CORES))],
                    ins=[arn_in.opt()], outs=[arn_out.opt()])
                nsum = p_in.tile([P, NB], F32, tag="nsum", name="nsum")
                nc.sync.dma_start(nsum[:], arn_out[:])
                rec8 = p_in.tile([P, NB], F32, tag="rec8", name="rec8")
                nc.vector.reciprocal(rec8[:], nsum[:])
                # sqrt((1/norm^2) * (1/T^2)) = 1/(T*norm)
                nc.scalar.activation(s_tile[:], rec8[:], AF.Sqrt,
                                     scale=1.0 / (TEMP * TEMP))
                _emit_gather(1)

            # ---- phase 3: similarities, exp, denominators, argmax
            with tc.tile_pool(name="sims", bufs=7) as p_sims, \
                 tc.tile_pool(name="pt", bufs=2) as p_pt, \
                 tc.tile_pool(name="small", bufs=2) as p_sm:
                for g, (g0, GB) in enumerate(GROUPS):
                    exp_t = [p_sims.tile([P, KS], F32, tag="exp",
                                         name=f"exp_g{g}_{bl}")
                             for bl in range(GB)]
                    dsum_t = [p_sm.tile([P, NJ], F32, tag=f"dsum{bl}",
                                        name=f"dsum{bl}")
                              for bl in range(GB)]
                    # per-j-block top-8 maxima, folded at group end — keeps
                    # the big MAX8 scans off the post-matmul critical path
                    mblk_t = [p_sm.tile([P, 8 * NJ], F32, tag=f"mblk{bl}",
                                        name=f"mblk{bl}")
                              for bl in range(GB)]
                    for jc in range(NJ // JC):
                        pth, ptl = [], []
                        cs = slice(jc * JC * NBLK, (jc + 1) * JC * NBLK)
                        for e in range(NE):
                            t = p_pt.tile([P, JC * NBLK], BF16, tag=f"pth{e}",
                                          name=f"pth{e}")
                            nc.sync.dma_start(t[:], protT_hi[e * P:(e + 1) * P, cs])
                            pth.append(t)
                            t = p_pt.tile([P, JC * NBLK], BF16, tag=f"ptl{e}",
                                          name=f"ptl{e}")
                            nc.sync.dma_start(t[:], protT_lo[e * P:(e + 1) * P, cs])
                            ptl.append(t)
                        for ji in range(JC):
                            j = jc * JC + ji
                            js = slice(ji * NBLK, (ji + 1) * NBLK)
                            for bl in range(GB):
                                bt = g0 + bl
                                bs = slice(bt * P, (bt + 1) * P)
                                ps = psum.tile([P, NBLK], F32, tag="ps",
                                               name="ps")
                                for e in range(NE):
                                    nc.tensor.matmul(ps[:],
                                                     lhsT=pj_hi[e][:, bs],
                                                     rhs=pth[e][:, js],
                                                     start=(e == 0), stop=False)
                                    nc.tensor.matmul(ps[:],
                                                     lhsT=pj_hi[e][:, bs],
                                                     rhs=ptl[e][:, js],
                                                     start=False, stop=False)
                                    nc.tensor.matmul(ps[:],
                                                     lhsT=pj_lo[e][:, bs],
                                                     rhs=pth[e][:, js],
                                                     start=False,
                                                     stop=(e == NE - 1))
                                nc.scalar.activation(
                                    exp_t[bl][:, j * NBLK:(j + 1) * NBLK],
                                    ps[:], AF.Exp,
                                    scale=s_tile[:, bt:bt + 1],
                                    accum_out=dsum_t[bl][:, j:j + 1])
                                nc.vector.max(
                                    mblk_t[bl][:, j * 8:(j + 1) * 8],
                                    exp_t[bl][:, j * NBLK:(j + 1) * NBLK])

                    # local argmax on the unnormalized numerators (argmax is
                    # invariant to the positive per-row normalization, and
                    # the values are cross-core comparable since every core
                    # uses identical row scales) — runs before the collective.
                    for bl in range(GB):
                        bt = g0 + bl
                        m8 = p_sm.tile([P, 8], F32, tag="m8", name="m8")
                        i8 = p_sm.tile([P, 8], U32, tag="i8", name="i8")
                        nc.vector.max(m8[:], mblk_t[bl][:])
                        nc.vector.max_index(i8[:], m8[:], exp_t[bl][:])
                        nc.sync.dma_start(lmax[bt * P:(bt + 1) * P, :], m8[:])
                        nc.sync.dma_start(lidx[bt * P:(bt + 1) * P, :], i8[:])

                    den = p_sm.tile([P, 4], F32, tag="den", name="den")
                    for bl in range(GB):
                        nc.vector.tensor_reduce(den[:, bl:bl + 1], dsum_t[bl][:],
                                                axis=AX.X, op=ALU.add)
                    cin = dram.tile([P, 4], F32, tag="cin", name="cin")
                    cout = dram.tile([P, 4], F32, tag="cout", name="cout")
                    nc.sync.dma_start(cin[:], den[:])
                    nc.gpsimd.collective_compute(
                        "AllReduce", ALU.add,
                        replica_groups=[list(range(N_CORES))],
                        ins=[cin.opt()], outs=[cout.opt()])
                    gd = p_sm.tile([P, 4], F32, tag="gd", name="gd")
                    nc.sync.dma_start(gd[:], cout[:])
                    rd = p_sm.tile([P, 4], F32, tag="rd", name="rd")
                    nc.vector.reciprocal(rd[:], gd[:])

                    for bl in range(GB):
                        bt = g0 + bl
                        # normalization scale on the scalar engine — the
                        # vector engine is running the FIND_INDEX8 scans
                        nc.scalar.mul(exp_t[bl][:], exp_t[bl][:],
                                      rd[:, bl:bl + 1])
                        nc.sync.dma_start(act[bt * P:(bt + 1) * P, :],
                                          exp_t[bl][:])

    nc.compile()
    _CACHE["nc"] = nc
    return nc


def _split_bf16(x):
    hi = x.astype(ml_dtypes.bfloat16)
    lo = (x - hi.astype(np.float32)).astype(ml_dtypes.bfloat16)
    return np.ascontiguousarray(hi), np.ascontiguousarray(lo)


def kernel(features, W, b, prototypes, _run_kwargs=None):
    nc = _build()

    featT = np.asarray(features, dtype=np.float32).T
    WT = np.asarray(W, dtype=np.float32).T
    bvec = np.asarray(b, dtype=np.float32)
    prototypes = np.asarray(prototypes, dtype=np.float32)

    fh, fl = _split_bf16(featT)

    in_maps = []
    for c in range(N_CORES):
        ph, pl = _split_bf16(prototypes[c * KS:(c + 1) * KS].T)
        wsh, wsl = _split_bf16(WT[:, c * P:(c + 1) * P])
        in_maps.append({"featT_hi": fh, "featT_lo": fl,
                        "WTs_hi": wsh, "WTs_lo": wsl,
                        "bvec": np.ascontiguousarray(bvec[c * P:(c + 1) * P]),
                        "protT_hi": ph, "protT_lo": pl})

    res = bass_utils.run_bass_kernel_spmd(
        nc, in_maps, core_ids=list(range(N_CORES)), **(_run_kwargs or {}))
    if _run_kwargs:
        _CACHE["last_result"] = res

    act = np.concatenate([res.results[c]["act"] for c in range(N_CORES)],
                         axis=1)
    lmax = np.stack([res.results[c]["lmax"][:, 0] for c in range(N_CORES)])
    lidx = np.stack([res.results[c]["lidx"][:, 0] for c in range(N_CORES)])
    best_core = np.argmax(lmax, axis=0)                       # [B]
    rows = np.arange(B)
    best_idx = (best_core * KS + lidx[best_core, rows]).astype(np.int32)
    return act, best_idx
